# revision 1
# baseline (speedup 1.0000x reference)
"""BartAttention (focused-attention variant) Trainium2 Bass kernel.

Problem (hardcoded): B=2, T=2048, D=1024, H=16 heads, hd=64.
  q = (h @ Wq.T + bq) * hd**-0.5 ; k = h @ Wk.T + bk ; v = h @ Wv.T + bv
  scores = q @ k.T per head ; e = f * exp(scores) ; attn = e / rowsum(e)
  out = (attn @ v) @ Wo.T + bo

Sharding over 8 cores: batch (2) x head-group (4 groups of 4 heads).
Each core computes its heads' QKV, attention, and a partial out-projection
(contraction over its 256 d-columns of Wo); host sums the 4 partials per
batch and adds bo.

On-device layout (per core):
  hT   [1024, 2048] bf16   hidden.T               (c on partitions)
  qT,kT [256, 2048] bf16   q.T / k.T              (head*hd on partitions)
  v    [2048, 4, 65] bf16  v per head + ones col  (s on partitions)
  scores computed transposed: sT[s,t] = k @ q.T so that e=f.T*exp(sT) has
  s on partitions, which is the contraction dim of the PV matmul.
  PV: outT_aug[65, t] = [v | 1].T @ e  -> row 64 = rowsum(e) per t (exact fp32).
  out-proj: final[t, m] = outT.T @ Wo_slice.T, scaled per head by 1/rowsum
  (broadcast via K=1 fp32r matmul before the multiply).

Schedule notes: QKV is emitted t-chunk-major following the hT DMA chunks, with
the first head-pair's attention interleaved so ACT (exp) starts early; the
per-head-pair PV accumulators use 2 PSUM banks, freed quickly by a deferred
normalization (recip + raw copy first, broadcast-multiply later).
"""

import numpy as np
import ml_dtypes

import concourse.bass as bass
import concourse.bacc as bacc
import concourse.mybir as mybir
from concourse.tile import TileContext
from concourse.bass_utils import run_bass_kernel_spmd

BF16 = mybir.dt.bfloat16
F32 = mybir.dt.float32
F32R = mybir.dt.float32r
AF = mybir.ActivationFunctionType

B, T, D = 2, 2048, 1024
H, HD = 16, 64
HG = 4               # heads per core
R = HG * HD          # 256 d-rows per core
SCALING = HD ** -0.5
N_CORES = 8

P = 128
KT = D // P          # 8 k-tiles for QKV contraction
MT = R // P          # 2 m-tiles of qT/kT rows
NCH = T // 512       # 4 chunks of 512 along t
ST = T // P          # 16 s-tiles

GPS_MOD = 0          # every GPS_MOD-th f-multiply goes to GPSIMD (0 = none)
E_BUFS = 6


def build_bass():
    nc = bacc.Bacc()

    hT_d = nc.declare_dram_parameter("hT", [D, T], BF16, isOutput=False)
    fT_d = nc.declare_dram_parameter("fT", [T, T], BF16, isOutput=False)
    wqT_d = nc.declare_dram_parameter("wqT", [D, R], BF16, isOutput=False)
    wkT_d = nc.declare_dram_parameter("wkT", [D, R], BF16, isOutput=False)
    wvT_d = nc.declare_dram_parameter("wvT", [D, R], BF16, isOutput=False)
    woT_d = nc.declare_dram_parameter("woT", [R, D], BF16, isOutput=False)
    bq_d = nc.declare_dram_parameter("bq", [R, 1], F32, isOutput=False)
    bk_d = nc.declare_dram_parameter("bk", [R, 1], F32, isOutput=False)
    bv_d = nc.declare_dram_parameter("bv", [1, R], BF16, isOutput=False)
    out_d = nc.declare_dram_parameter("out_partial", [T, D], F32, isOutput=True)

    with TileContext(nc) as tc:
        with (
            nc.allow_low_precision(reason="bf16/f32r pipeline is intentional"),
            tc.tile_pool(name="sb", bufs=1) as sb,
            tc.tile_pool(name="ps", bufs=1, space="PSUM") as ps,
        ):
            # ---- persistent SBUF tensors ----
            hT = sb.tile([P, KT, T], BF16)
            wqT = sb.tile([P, KT, R], BF16)
            wkT = sb.tile([P, KT, R], BF16)
            wvT = sb.tile([P, KT, R], BF16)
            woT = sb.tile([P, MT, D], BF16)
            bq = sb.tile([P, MT], F32)
            bk = sb.tile([P, MT], F32)
            bv = sb.tile([1, R], BF16)
            ones_r = sb.tile([1, P], BF16)     # K=1 lhsT for v-bias matmul
            ones64 = sb.tile([1, HD], F32R)    # K=1 lhsT for rowsum broadcast
            qT = sb.tile([P, MT, T], BF16)
            kT = sb.tile([P, MT, T], BF16)
            vsb = sb.tile([P, ST, HG, HD + 1], BF16)
            po = sb.tile([P, MT, T], BF16)     # scaled outT, out-proj lhsT
            eu01 = sb.tile([P, ST, 1024], BF16)  # unit (0,1) e, PV deferred

            nc.sync.dma_start(wqT[:], wqT_d.rearrange("(k p) r -> p k r", p=P))
            nc.sync.dma_start(wkT[:], wkT_d.rearrange("(k p) r -> p k r", p=P))
            nc.sync.dma_start(bq[:], bq_d.rearrange("(m p) one -> p (m one)", p=P))
            nc.sync.dma_start(bk[:], bk_d.rearrange("(m p) one -> p (m one)", p=P))
            nc.sync.dma_start(bv[:], bv_d[:])
            ones64_f32 = sb.tile([1, HD], F32)
            nc.vector.memset(ones_r[:], 1.0)
            nc.vector.memset(ones64_f32[:], 1.0)
            nc.vector.tensor_copy(ones64[:], ones64_f32[:])
            nc.vector.memset(vsb[:, :, :, HD : HD + 1], 1.0)

            hT_r = hT_d.rearrange("(k p) t -> p k t", p=P)
            mul_i = [0]

            def qkv_chunk(n):
                """QKV outputs for t-columns [n*512, (n+1)*512)."""
                nsl = slice(n * 512, (n + 1) * 512)
                for kk in range(0, KT, 2):
                    nc.sync.dma_start(
                        hT[:, kk : kk + 2, nsl], hT_r[:, kk : kk + 2, nsl]
                    )
                for w_sb, b_sb, o_sb in ((wqT, bq, qT), (wkT, bk, kT)):
                    for m in range(MT):
                        acc = ps.tile([P, 512], F32, tag="pv", bufs=4,
                                      name=f"qkacc_{n}_{m}")
                        for k in range(KT):
                            nc.tensor.matmul(
                                acc[:],
                                w_sb[:, k, m * P : (m + 1) * P],
                                hT[:, k, nsl],
                                start=(k == 0),
                                stop=(k == KT - 1),
                            )
                        nc.vector.tensor_scalar_add(
                            o_sb[:, m, nsl], acc[:], b_sb[:, m : m + 1]
                        )
                if n == 0:
                    nc.sync.dma_start(
                        wvT[:], wvT_d.rearrange("(k p) r -> p k r", p=P)
                    )
                for s in range(4 * n, 4 * n + 4):
                    acc = ps.tile([P, R], F32, tag="pv", bufs=4, name=f"vacc_{s}")
                    for k in range(KT):
                        nc.tensor.matmul(
                            acc[:],
                            hT[:, k, s * P : (s + 1) * P],
                            wvT[:, k, :],
                            start=(k == 0),
                            stop=False,
                        )
                    nc.tensor.matmul(acc[:], ones_r[:], bv[:], start=False, stop=True)
                    nc.vector.tensor_copy(
                        vsb[:, s, :, 0:HD],
                        acc[:].rearrange("p (h d) -> p h d", h=HG),
                    )

            def ft_load(tch, st):
                ftt = ft_tiles[tch]
                nc.sync.dma_start(
                    ftt[:, st, :],
                    fT_d[st * P : (st + 1) * P, tch * 512 : (tch + 1) * 512],
                )

            def attn_steps(tch, j, pvp, st_range):
                """Scores/exp/f-mul/PV for head pair j over st_range."""
                tsl = slice(tch * 512, (tch + 1) * 512)
                ftt = ft_tiles[tch]
                for st in st_range:
                    ssl = slice(st * P, (st + 1) * P)
                    sc = ps.tile([P, 1024], F32, tag="sc", bufs=2,
                                 name=f"sc_{tch}_{j}_{st}")
                    e = sb.tile([P, 1024], BF16, tag="e", bufs=E_BUFS,
                                name=f"e_{tch}_{j}_{st}")
                    for a in range(2):
                        rows = slice(a * HD, (a + 1) * HD)
                        nc.tensor.matmul(
                            sc[:, a * 512 : (a + 1) * 512],
                            kT[rows, j, ssl],
                            qT[rows, j, tsl],
                            start=True,
                            stop=True,
                        )
                    nc.scalar.activation(e[:], sc[:], AF.Exp)
                    for a in range(2):
                        half = slice(a * 512, (a + 1) * 512)
                        mul_i[0] += 1
                        use_gps = GPS_MOD and (mul_i[0] % GPS_MOD == 0)
                        eng = nc.gpsimd if use_gps else nc.vector
                        eng.tensor_mul(e[:, half], e[:, half], ftt[:, st, :])
                        nc.tensor.matmul(
                            pvp[a][:],
                            vsb[:, st, 2 * j + a, :],
                            e[:, half],
                            start=(st == 0),
                            stop=(st == ST - 1),
                        )

            pending_norms = []

            def norm_fast(tch, j, pvp):
                """Free the PSUM accumulators quickly: reciprocal of the
                rowsum row + raw copy-out; the scale multiply is deferred."""
                for a in range(2):
                    h = 2 * j + a
                    recip = sb.tile([1, 512], F32R, tag="recip", bufs=4,
                                    name=f"recip_{tch}_{h}")
                    praw = sb.tile([HD, 512], BF16, tag="praw", bufs=4,
                                   name=f"praw_{tch}_{h}")
                    nc.vector.reciprocal(recip[:], pvp[a][HD : HD + 1, :])
                    # praw on ACT: runs concurrently with the DVE recips, so
                    # the PSUM pair frees in half the time at pair boundaries
                    nc.scalar.copy(praw[:], pvp[a][0:HD, :])
                    pending_norms.append((tch, h, recip, praw))

            def norm_defer():
                """Broadcast 1/rowsum (K=1 f32r matmul) and scale into po."""
                while pending_norms:
                    tch, h, recip, praw = pending_norms.pop(0)
                    bcs = sb.tile([HD, 512], BF16, tag="bcs", bufs=4,
                                  name=f"bcs_{tch}_{h}")
                    bcp = ps.tile([HD, 512], F32, tag="sc", bufs=2,
                                  name=f"bcp_{tch}_{h}")
                    nc.tensor.matmul(bcp[:], ones64[:], recip[:], start=True, stop=True)
                    nc.vector.tensor_copy(bcs[:], bcp[:])
                    nc.vector.tensor_mul(
                        po[(h % 2) * HD : (h % 2) * HD + HD, h // 2,
                           tch * 512 : (tch + 1) * 512],
                        praw[:],
                        bcs[:],
                    )

            ft_tiles = {}

            def u01_scores(st_range):
                """Unit (0,1) scores/exp/f-mul during QKV; e kept in SBUF,
                PV deferred so no PSUM accumulators are held early."""
                for st in st_range:
                    ssl = slice(st * P, (st + 1) * P)
                    sc = ps.tile([P, 1024], F32, tag="sc", bufs=2,
                                 name=f"sc01_{st}")
                    for a in range(2):
                        rows = slice(a * HD, (a + 1) * HD)
                        nc.tensor.matmul(
                            sc[:, a * 512 : (a + 1) * 512],
                            kT[rows, 1, ssl],
                            qT[rows, 1, 0:512],
                            start=True,
                            stop=True,
                        )
                    nc.scalar.activation(eu01[:, st, :], sc[:], AF.Exp)
                    for a in range(2):
                        nc.vector.tensor_mul(
                            eu01[:, st, a * 512 : (a + 1) * 512],
                            eu01[:, st, a * 512 : (a + 1) * 512],
                            ft_tiles[0][:, st, :],
                        )

            def u01_pv_step(pvp, st):
                for a in range(2):
                    nc.tensor.matmul(
                        pvp[a][:],
                        vsb[:, st, 2 + a, :],
                        eu01[:, st, a * 512 : (a + 1) * 512],
                        start=(st == 0),
                        stop=(st == ST - 1),
                    )

            def outproj_unit(u):
                tt, n = divmod(u, 2)
                fin = ps.tile([P, 512], F32, tag="pv", bufs=4, name=f"fin_{tt}_{n}")
                osb = sb.tile([P, 512], F32, tag="osb", bufs=3, name=f"osb_{tt}_{n}")
                for j in range(MT):
                    nc.tensor.matmul(
                        fin[:],
                        po[:, j, tt * P : (tt + 1) * P],
                        woT[:, j, n * 512 : (n + 1) * 512],
                        start=(j == 0),
                        stop=(j == MT - 1),
                    )
                if tt % 2 == 0:
                    nc.scalar.copy(osb[:], fin[:])
                else:
                    nc.vector.tensor_copy(osb[:], fin[:])
                nc.sync.dma_start(
                    out_d[tt * P : (tt + 1) * P, n * 512 : (n + 1) * 512], osb[:]
                )

            def new_pv_pair(tch, j):
                return [ps.tile([HD + 1, 512], F32, tag="pv", bufs=4,
                                name=f"pv_{tch}_{j}_{a}") for a in range(2)]

            def new_ft(tch):
                ft_tiles[tch] = sb.tile([P, ST, 512], BF16,
                                        tag=f"ft{tch % 2}", bufs=1, name=f"ft_t{tch}")

            # ---- emission ----
            # u(0,0) rides along the QKV chunks; the remaining 7 (tch, j)
            # units run as overlapping pairs (two independent
            # scores->exp->mul->PV chains keep every engine fed); the last
            # unit interleaves with the out-projection of finished t-chunks.
            new_ft(0)
            pv00 = new_pv_pair(0, 0)
            for n in range(NCH):
                qkv_chunk(n)
                for st in range(4 * n, 4 * n + 4):
                    ft_load(0, st)
                attn_steps(0, 0, pv00, range(4 * n, 4 * n + 4))
                u01_scores(range(4 * n, 4 * n + 4))
            norm_fast(0, 0, pv00)
            nc.sync.dma_start(woT[:], woT_d.rearrange("(m p) d -> p m d", p=P))

            pv01, pv10 = new_pv_pair(0, 1), new_pv_pair(1, 0)
            new_ft(1)
            for st in range(ST):
                ft_load(1, st)
            for st in range(ST):
                u01_pv_step(pv01, st)
                attn_steps(1, 0, pv10, (st,))
                if st == 2:
                    norm_defer()
            norm_fast(0, 1, pv01)
            norm_fast(1, 0, pv10)

            for ua, ub in (((1, 1), (2, 0)), ((2, 1), (3, 0))):
                for tch in (ua[0], ub[0]):
                    if tch not in ft_tiles:
                        new_ft(tch)
                        for st in range(ST):
                            ft_load(tch, st)
                pva, pvb = new_pv_pair(*ua), new_pv_pair(*ub)
                for st in range(ST):
                    attn_steps(*ua, pva, (st,))
                    attn_steps(*ub, pvb, (st,))
                    if st == 2:
                        norm_defer()
                norm_fast(*ua, pva)
                norm_fast(*ub, pvb)

            pv31 = new_pv_pair(3, 1)
            for st in range(ST):
                attn_steps(3, 1, pv31, (st,))
                if st == 2:
                    norm_defer()
                if st < 12:
                    outproj_unit(2 * st)
                    outproj_unit(2 * st + 1)
            norm_fast(3, 1, pv31)
            norm_defer()
            for u in range(24, 32):
                outproj_unit(u)

    return nc


_NC = None
_LAST_RESULT = None


def _get_nc():
    global _NC
    if _NC is None:
        _NC = build_bass()
        if not _NC.is_finalized():
            _NC.finalize()
    return _NC


def kernel(hidden_states, focused_attention, Wq, bq, Wk, bk, Wv, bv, Wo, bo):
    bf = ml_dtypes.bfloat16
    hT = [np.ascontiguousarray(hidden_states[b].T).astype(bf) for b in range(B)]
    fT = [np.ascontiguousarray(focused_attention[b].T).astype(bf) for b in range(B)]

    in_maps = []
    for c in range(N_CORES):
        b, g = divmod(c, 4)
        rows = slice(g * R, (g + 1) * R)
        in_maps.append({
            "hT": hT[b],
            "fT": fT[b],
            "wqT": np.ascontiguousarray((Wq[rows] * SCALING).T).astype(bf),
            "wkT": np.ascontiguousarray(Wk[rows].T).astype(bf),
            "wvT": np.ascontiguousarray(Wv[rows].T).astype(bf),
            "woT": np.ascontiguousarray(Wo[:, rows].T).astype(bf),
            "bq": np.ascontiguousarray((bq[rows] * SCALING)[:, None]).astype(np.float32),
            "bk": np.ascontiguousarray(bk[rows][:, None]).astype(np.float32),
            "bv": np.ascontiguousarray(bv[rows][None, :]).astype(bf),
        })

    res = run_bass_kernel_spmd(_get_nc(), in_maps, list(range(N_CORES)))
    global _LAST_RESULT
    _LAST_RESULT = res
    out = np.zeros((B, T, D), dtype=np.float32)
    for c in range(N_CORES):
        out[c // 4] += res.results[c]["out_partial"]
    out += np.asarray(bo, dtype=np.float32)[None, None, :]
    return out



# revision 22
# speedup vs baseline: 1.1959x; 1.1959x over previous
"""BartAttention (focused-attention variant) Trainium2 Bass kernel, v3.

Problem (hardcoded): B=2, T=2048, D=1024, H=16 heads, hd=64.
  q = (h @ Wq.T + bq) * hd**-0.5 ; k = h @ Wk.T + bk ; v = h @ Wv.T + bv
  scores = q @ k.T per head ; e = f * exp(scores) ; attn = e / rowsum(e)
  out = (attn @ v) @ Wo.T + bo

Sharding over 8 cores: batch (2) x head-group (4 groups of 4 heads).
Each core computes its heads' QKV, attention, and a partial out-projection
(contraction over its 256 d-columns of Wo); host sums the 4 partials per
batch and adds bo.

Design: scores are computed transposed ([s,t], s on partitions) in 2-s-tile
PSUM groups; exp on ACT and the f-multiply on DVE run per group into a ring
of e tiles. PV consumes e as the stationary operand with rhs = [v | 1]:
out [t=128, hd+1] per head (half the PE rows of the outT formulation),
rowsum free in column 64; normalization is a per-partition tensor_scalar
multiply. Normalized head-pair tiles [t=128, 128] are PE-transposed into
the out-projection lhsT layout. PSUM evictions (qk bias adds, v copies,
poT copies, out-proj copies) run on the otherwise-idle GPSIMD/Pool engine.

Dependency-tracking rule this layout exploits: slice-disjoint uses of one
tile still serialize (conservative tracking), so every pipelined buffer
(sc groups, e groups, ft pairs, fin) is its own tag-ring tile.

PSUM map (8 banks): sc ring 2x2 banks + 2 PV banks + fin ring 2x1;
stage 1 reuses the PV/fin banks for QKV accumulators.
"""

import numpy as np
import ml_dtypes

import concourse.bass as bass
import concourse.bacc as bacc
import concourse.mybir as mybir
from concourse.tile import TileContext
from concourse.bass_utils import run_bass_kernel_spmd

BF16 = mybir.dt.bfloat16
F32 = mybir.dt.float32
AF = mybir.ActivationFunctionType

B, T, D = 2, 2048, 1024
H, HD = 16, 64
HG = 4               # heads per core
R = HG * HD          # 256 d-rows per core
SCALING = HD ** -0.5
N_CORES = 8

P = 128
KT = D // P          # 8 k-tiles for QKV contraction
MT = R // P          # 2 m-tiles (head pairs)
NCH = T // 512       # 4 t-chunks of 512
ST = T // P          # 16 s-tiles
NG = ST // 2         # 8 score groups (2 s-tiles each) per (tchunk, head)

E_BUFS = 44
FT_BUFS = 16


def build_bass():
    nc = bacc.Bacc()

    hT_d = nc.declare_dram_parameter("hT", [D, T], BF16, isOutput=False)
    fT_d = nc.declare_dram_parameter("fT", [T, T], BF16, isOutput=False)
    wqT_d = nc.declare_dram_parameter("wqT", [D, R], BF16, isOutput=False)
    wkT_d = nc.declare_dram_parameter("wkT", [D, R], BF16, isOutput=False)
    wvT_d = nc.declare_dram_parameter("wvT", [D, R], BF16, isOutput=False)
    woT_d = nc.declare_dram_parameter("woT", [R, D], BF16, isOutput=False)
    bq_d = nc.declare_dram_parameter("bq", [R, 1], F32, isOutput=False)
    bk_d = nc.declare_dram_parameter("bk", [R, 1], F32, isOutput=False)
    bv_d = nc.declare_dram_parameter("bv", [1, R], BF16, isOutput=False)
    ident_d = nc.declare_dram_parameter("ident", [P, P], BF16, isOutput=False)
    out_d = nc.declare_dram_parameter("out_partial", [T, D], F32, isOutput=True)

    with TileContext(nc) as tc:
        with (
            nc.allow_low_precision(reason="bf16 pipeline is intentional"),
            tc.tile_pool(name="sb", bufs=1) as sb,
            tc.tile_pool(name="ps", bufs=1, space="PSUM") as ps,
        ):
            # ---- persistent SBUF tensors ----
            wqT = sb.tile([P, KT, R], BF16)
            wkT = sb.tile([P, KT, R], BF16)
            wvT = sb.tile([P, KT, R], BF16)
            woT = sb.tile([P, MT, D], BF16)
            bq = sb.tile([P, MT], F32)
            bk = sb.tile([P, MT], F32)
            bv = sb.tile([1, R], BF16)
            ones_r = sb.tile([1, P], BF16)       # K=1 lhsT for v-bias matmul
            ident = sb.tile([P, P], BF16)        # PE transpose identity
            qT = sb.tile([P, MT, T], BF16)
            kT = sb.tile([P, MT, T], BF16)
            vsb = sb.tile([P, ST, HG, HD + 1], BF16)
            po = sb.tile([P, MT, T], BF16)       # out-proj lhsT

            # ---- PSUM (8 banks): sc ring 2x2 + pvA + pvB + fin ring 2x1
            pvA = ps.tile([P, 512], F32, name="pvA")
            pvB = ps.tile([P, 512], F32, name="pvB")

            hT_r = hT_d.rearrange("(k p) t -> p k t", p=P)
            wq_r = wqT_d.rearrange("(k p) r -> p k r", p=P)
            wk_r = wkT_d.rearrange("(k p) r -> p k r", p=P)
            hT_tiles = {}

            def hT_tile(n):
                if n not in hT_tiles:
                    hT_tiles[n] = sb.tile([P, KT, 512], BF16, tag="hT", bufs=3,
                                          name=f"hT_{n}")
                return hT_tiles[n]

            # critical-path first: wq/wk + hT chunk 0 in halves so the first
            # projection chains can start on the low k-tiles
            h0 = hT_tile(0)
            nc.sync.dma_start(wqT[:, 0:4, :], wq_r[:, 0:4, :])
            nc.sync.dma_start(h0[:, 0:4, :], hT_r[:, 0:4, 0:512])
            nc.sync.dma_start(wkT[:, 0:4, :], wk_r[:, 0:4, :])
            nc.sync.dma_start(wqT[:, 4:8, :], wq_r[:, 4:8, :])
            nc.sync.dma_start(h0[:, 4:8, :], hT_r[:, 4:8, 0:512])
            nc.sync.dma_start(wkT[:, 4:8, :], wk_r[:, 4:8, :])
            nc.sync.dma_start(bq[:], bq_d.rearrange("(m p) one -> p (m one)", p=P))
            nc.sync.dma_start(bk[:], bk_d.rearrange("(m p) one -> p (m one)", p=P))
            nc.sync.dma_start(bv[:], bv_d[:])
            nc.sync.dma_start(ident[:], ident_d[:])
            nc.vector.memset(ones_r[:], 1.0)
            nc.vector.memset(vsb[:, :, :, HD : HD + 1], 1.0)

            # ---------- helpers ----------
            ft_tiles = {}
            ft_i = [0]

            def ft_load(tch, pr):
                """Load f.T s-rows [pr*256,(pr+1)*256) x t-chunk as [P,2,512]."""
                t = sb.tile([P, 2, 512], BF16, tag="ft", bufs=FT_BUFS,
                            name=f"ft_{ft_i[0]}")
                ft_i[0] += 1
                nc.sync.dma_start(
                    t[:],
                    fT_d[pr * 256 : (pr + 1) * 256,
                         tch * 512 : (tch + 1) * 512].rearrange(
                        "(two p) t -> p two t", p=P),
                )
                ft_tiles[(tch, pr)] = t

            fin_i = [0]

            def fin_tile():
                t = ps.tile([P, 512], F32, tag="fin", bufs=2,
                            name=f"fin_{fin_i[0]}")
                fin_i[0] += 1
                return t

            qkv_ring = [pvA, pvB]
            qkv_i = [0]

            def qk_proj(n, w_sb, b_sb, o_sb, m):
                nsl = slice(n * 512, (n + 1) * 512)
                ht = hT_tile(n)
                acc = qkv_ring[qkv_i[0] % 3] if qkv_i[0] % 3 != 2 else fin_tile()
                qkv_i[0] += 1
                for k in range(KT):
                    nc.tensor.matmul(
                        acc[:],
                        w_sb[:, k, m * P : (m + 1) * P],
                        ht[:, k, :],
                        start=(k == 0),
                        stop=(k == KT - 1),
                    )
                nc.vector.tensor_scalar_add(o_sb[:, m, nsl], acc[:], b_sb[:, m : m + 1])

            def v_proj(s):
                ht = hT_tile(s // 4)
                acc = qkv_ring[qkv_i[0] % 3] if qkv_i[0] % 3 != 2 else fin_tile()
                qkv_i[0] += 1
                for k in range(KT):
                    nc.tensor.matmul(
                        acc[:, 0:R],
                        ht[:, k, (s % 4) * P : (s % 4 + 1) * P],
                        wvT[:, k, :],
                        start=(k == 0),
                        stop=False,
                    )
                nc.tensor.matmul(acc[:, 0:R], ones_r[:], bv[:], start=False, stop=True)
                nc.vector.tensor_copy(
                    vsb[:, s, :, 0:HD],
                    acc[:, 0:R].rearrange("p (h d) -> p h d", h=HG),
                )

            e_tiles = {}
            sc_i = [0]

            def score_group(tc_i, h, g):
                """Scores+exp+fmul for s-tiles {2g, 2g+1} of (tchunk, head)."""
                j, a = divmod(h, 2)
                rows = slice(a * HD, (a + 1) * HD)
                tsl = slice(tc_i * 512, (tc_i + 1) * 512)
                scg = ps.tile([P, 2, 512], F32, tag="sc", bufs=2,
                              name=f"sc_{sc_i[0]}")
                et = sb.tile([P, 2, 512], BF16, tag="e", bufs=E_BUFS,
                             name=f"e_{sc_i[0]}")
                sc_i[0] += 1
                for i in range(2):
                    st = 2 * g + i
                    nc.tensor.matmul(
                        scg[:, i, :],
                        kT[rows, j, st * P : (st + 1) * P],
                        qT[rows, j, tsl],
                        start=True,
                        stop=True,
                    )
                nc.scalar.activation(et[:], scg[:], AF.Exp)
                eng = nc.gpsimd if sc_i[0] % 3 == 0 else nc.vector
                eng.tensor_mul(et[:], et[:], ft_tiles[(tc_i, g)][:])
                e_tiles[(tc_i, h, g)] = et

            def score_head(tc_i, h, g0=0):
                for g in range(g0, NG):
                    if (tc_i, h, g) in e_tiles:
                        continue
                    score_group(tc_i, h, g)
                    yield

            recip_i = [0]

            def pv_tblock(tc_i, p, b, pv):
                """PV chains for head pair p, t-block b of tchunk tc_i, into
                pv bank cols [0:65],[65:130]; norm + transpose + evict to po."""
                for h_in in range(2):
                    h = 2 * p + h_in
                    c0 = h_in * (HD + 1)
                    for st in range(ST):
                        nc.tensor.matmul(
                            pv[:, c0 : c0 + HD + 1],
                            e_tiles[(tc_i, h, st // 2)][:, st % 2,
                                                        b * P : (b + 1) * P],
                            vsb[:, st, h, :],
                            start=(st == 0),
                            stop=(st == ST - 1),
                        )
                recip = sb.tile([P, 2], F32, tag="recip", bufs=4,
                                name=f"recip_{recip_i[0]}")
                ob = sb.tile([P, P], BF16, tag="ob", bufs=4,
                             name=f"ob_{recip_i[0]}")
                recip_i[0] += 1
                nc.vector.reciprocal(
                    recip[:],
                    pv[:, 0 : 2 * (HD + 1)].rearrange(
                        "p (h n) -> p h n", n=HD + 1)[:, :, HD],
                )
                for h_in in range(2):
                    c0 = h_in * (HD + 1)
                    nc.vector.tensor_scalar_mul(
                        ob[:, h_in * HD : (h_in + 1) * HD],
                        pv[:, c0 : c0 + HD],
                        recip[:, h_in : h_in + 1],
                    )
                # transpose into a carve of the same pv bank (cols 256:320
                # fp32 = [128,128] bf16); the PV range [0:130] of this bank
                # is not reused until two t-blocks later.
                poT = pv[:, 256:320].bitcast(BF16)
                nc.tensor.transpose(poT, ob[:], ident[:])
                tt = tc_i * 4 + b
                nc.vector.tensor_copy(po[:, p, tt * P : (tt + 1) * P], poT)

            osb_i = [0]

            def outproj_unit(u):
                tt, nn = divmod(u, 2)
                fin = fin_tile()
                for j in range(MT):
                    nc.tensor.matmul(
                        fin[:],
                        po[:, j, tt * P : (tt + 1) * P],
                        woT[:, j, nn * 512 : (nn + 1) * 512],
                        start=(j == 0),
                        stop=(j == MT - 1),
                    )
                osb = sb.tile([P, 512], F32, tag="osb", bufs=3,
                              name=f"osb_{osb_i[0]}")
                osb_i[0] += 1
                nc.vector.tensor_copy(osb[:], fin[:])
                nc.sync.dma_start(
                    out_d[tt * P : (tt + 1) * P, nn * 512 : (nn + 1) * 512], osb[:]
                )

            # ---------- emission ----------
            # stage 1: QKV chunks with tchunk-0 score groups interleaved so
            # PE has filler work between dependent score/exp rounds.
            for pr in range(4):
                ft_load(0, pr)
            for n in range(NCH):
                if n + 1 < NCH:
                    nsl = slice((n + 1) * 512, (n + 2) * 512)
                    nc.sync.dma_start(hT_tile(n + 1)[:], hT_r[:, :, nsl])
                qk_proj(n, wqT, bq, qT, 0)
                if n >= 1:
                    # borrow tchunk-1 groups into the PE-bound stage-1 to
                    # feed ACT (windows are ACT-bound); paced one group per
                    # projection block so PE never hits the sc-ring wall
                    score_group(1, 0, n - 1)
                qk_proj(n, wqT, bq, qT, 1)
                if n >= 1:
                    score_group(1, 1, n - 1)
                qk_proj(n, wkT, bk, kT, 0)
                if n >= 1:
                    score_group(1, 2, n - 1)
                qk_proj(n, wkT, bk, kT, 1)
                if n >= 1:
                    score_group(1, 3, n - 1)
                if n == 0:
                    nc.sync.dma_start(
                        wvT[:], wvT_d.rearrange("(k p) r -> p k r", p=P)
                    )
                    for pr in range(4, 8):
                        ft_load(0, pr)
                    ft_load(1, 0)
                    ft_load(1, 1)
                if n == 1:
                    nc.sync.dma_start(
                        woT[:], woT_d.rearrange("(m p) d -> p m d", p=P)
                    )
                    ft_load(1, 2)
                    ft_load(1, 3)
                if n == 2:
                    for pr in range(4, 8):
                        ft_load(1, pr)
                # interleave v-projections with tchunk-0 score groups for the
                # s-tiles whose k just landed (groups 2n and 2n+1 per head)
                for i, s in enumerate(range(4 * n, 4 * n + 4)):
                    v_proj(s)
                    h = i
                    score_group(0, h, 2 * n)
                    score_group(0, h, 2 * n + 1)

            # stage 2: 4 windows. window c: PV(c) + scores(c+1) + outproj(c)
            # (out-proj lags pair-1 PV by one t-block within the window).
            pv_banks = [pvA, pvB]
            for c in range(NCH):
                gens = ([score_head(c + 1, h) for h in range(HG)]
                        if c + 1 < NCH else [])
                pv_units = [(c, p, b) for p in range(2) for b in range(4)]
                if c == 2:
                    pv_units += [(3, 0, b) for b in range(4)]
                elif c == 3:
                    pv_units = [(3, 1, b) for b in range(4)]
                ft_pref = list(range(8)) if c + 2 < NCH else []

                def draw_scores(k):
                    for _ in range(k):
                        while gens:
                            try:
                                next(gens[0])
                                break
                            except StopIteration:
                                gens.pop(0)
                        if not gens:
                            return

                for cc, p, b in pv_units:
                    pv_tblock(cc, p, b, pv_banks[b % 2])
                    if ft_pref:
                        ft_load(c + 2, ft_pref.pop(0))
                    draw_scores(4)
                    if p == 1 and b >= 1:
                        # po rows for t-block b-1 are complete (both pairs)
                        outproj_unit((cc * 4 + b - 1) * 2)
                        outproj_unit((cc * 4 + b - 1) * 2 + 1)
                if c < 3:
                    draw_scores(4 * HG * NG)
                outproj_unit((c * 4 + 3) * 2)
                outproj_unit((c * 4 + 3) * 2 + 1)

    return nc


_NC = None
_LAST_RESULT = None


def _get_nc():
    global _NC
    if _NC is None:
        _NC = build_bass()
        if not _NC.is_finalized():
            _NC.finalize()
    return _NC


def kernel(hidden_states, focused_attention, Wq, bq, Wk, bk, Wv, bv, Wo, bo):
    bf = ml_dtypes.bfloat16
    hT = [np.ascontiguousarray(hidden_states[b].T).astype(bf) for b in range(B)]
    fT = [np.ascontiguousarray(focused_attention[b].T).astype(bf) for b in range(B)]
    ident = np.eye(P, dtype=bf)

    in_maps = []
    for c in range(N_CORES):
        b, g = divmod(c, 4)
        rows = slice(g * R, (g + 1) * R)
        in_maps.append({
            "hT": hT[b],
            "fT": fT[b],
            "wqT": np.ascontiguousarray((Wq[rows] * SCALING).T).astype(bf),
            "wkT": np.ascontiguousarray(Wk[rows].T).astype(bf),
            "wvT": np.ascontiguousarray(Wv[rows].T).astype(bf),
            "woT": np.ascontiguousarray(Wo[:, rows].T).astype(bf),
            "bq": np.ascontiguousarray((bq[rows] * SCALING)[:, None]).astype(np.float32),
            "bk": np.ascontiguousarray(bk[rows][:, None]).astype(np.float32),
            "bv": np.ascontiguousarray(bv[rows][None, :]).astype(bf),
            "ident": ident,
        })

    res = run_bass_kernel_spmd(_get_nc(), in_maps, list(range(N_CORES)))
    global _LAST_RESULT
    _LAST_RESULT = res
    out = np.zeros((B, T, D), dtype=np.float32)
    for c in range(N_CORES):
        out[c // 4] += res.results[c]["out_partial"]
    out += np.asarray(bo, dtype=np.float32)[None, None, :]
    return out
